# revision 13
# baseline (speedup 1.0000x reference)
"""Causal self-attention Trainium2 Bass kernel (software-pipelined).

Problem: B=128, T=256, D=512, H=8 heads of 64. f32 in/out.
Sharding: data-parallel over batch - 16 batches per NeuronCore, weights
replicated, no collectives.

Matmul datapath in fp16 (1 col/cycle moving-operand stream at 2.4GHz);
PSUM accumulation fp32. The N=512 projection matmuls are the dominant
irreducible PE cost (~110us/core), so the kernel keeps the PE dense and
warm end to end (matmul-queue idle measured <1us per run):

  1. Cross-pair software pipelining: attention of pair p (small MMs with
     ACT/GpSimd dependencies in the softmax chain) is interleaved at
     issue time with the N=512 projection groups of pair p+1 and the
     output projection of pair p-2, so the PE FIFO always has
     independent work and the HAM clock gate never re-throttles.
     Per-slot PE order: [T(s-2), S(s), G, O(s-1), G].
  2. Deferring the output projection by two pairs keeps the last pair's
     queue full (8 real groups); zero-weight filler matmuls cover the
     remaining tail slots to hold K=8/8.
  3. Weights are passed partition-major ([128, sec, k, n]) so startup
     DMAs land 4KB contiguous lines; loads are spread across the three
     DMA-capable engine queues (scalar=W_Q, sync=W_K + xt prefetch,
     gpsimd=xt0 + small tensors + W_V/W_out) and warm-up matmuls on a
     zeroed tile run inside the DGE-latency window so the first real
     projection starts at full clock.
  4. Attention math per (head-pair, batch) slot: S^T both s-tiles into
     one [128,384] PSUM bank (causality), single exp on ACT,
     multiplicative causal mask on GpSimd, O-matmuls carry a ones
     column whose output is the softmax denominator (per-partition),
     one reciprocal + broadcast-multiply on DVE normalizes into a
     head-pair staging tile, PE transposes feed the out-projection.
     Scale 1/sqrt(hd) and the V-path bias are folded on the host.
"""

import numpy as np

B, T, D = 128, 256, 512
H, HD = 8, 64
NCORES = 8
BL = B // NCORES  # batches per core


def build_nc(bl=BL, num_devices=NCORES):
    from contextlib import ExitStack

    import concourse.bacc as bacc
    import concourse.tile as tile
    from concourse import mybir

    f32 = mybir.dt.float32
    f16 = mybir.dt.float16
    AF = mybir.ActivationFunctionType

    nc = bacc.Bacc(
        "TRN2",
        target_bir_lowering=False,
        debug=False,
        enable_asserts=False,
        num_devices=num_devices,
    )

    npairs = bl // 2

    xt_d = nc.dram_tensor("xt", [bl, 128, 4, 256], f16, kind="ExternalInput").ap()
    w_d = nc.dram_tensor("wqkv", [128, 3, 4, 512], f16, kind="ExternalInput").ap()
    wo_d = nc.dram_tensor("wout", [128, 4, D], f16, kind="ExternalInput").ap()
    bqk_d = nc.dram_tensor("bqk", [128, 8], f32, kind="ExternalInput").ap()
    bm_d = nc.dram_tensor("binm", [128, 128], f16, kind="ExternalInput").ap()
    id_d = nc.dram_tensor("ident", [128, 128], f16, kind="ExternalInput").ap()
    y_d = nc.dram_tensor("y", [bl, T, D], f16, kind="ExternalOutput").ap()

    with tile.TileContext(nc) as tc, ExitStack() as ctx:
        singles = ctx.enter_context(tc.tile_pool(name="singles", bufs=1))
        p_xt = ctx.enter_context(tc.tile_pool(name="p_xt", bufs=3))
        p_qkt = ctx.enter_context(tc.tile_pool(name="p_qkt", bufs=3))
        p_et = ctx.enter_context(tc.tile_pool(name="p_et", bufs=6))
        p_o = ctx.enter_context(tc.tile_pool(name="p_o", bufs=4))
        p_li = ctx.enter_context(tc.tile_pool(name="p_li", bufs=8))
        p_ot = ctx.enter_context(tc.tile_pool(name="p_ot", bufs=6))
        p_y = ctx.enter_context(tc.tile_pool(name="p_y", bufs=4))
        psA = ctx.enter_context(tc.tile_pool(name="psA", bufs=2, space="PSUM"))
        psB = ctx.enter_context(tc.tile_pool(name="psB", bufs=3, space="PSUM"))
        psC = ctx.enter_context(tc.tile_pool(name="psC", bufs=3, space="PSUM"))

        # ---- startup DMAs, spread across engine DGE queues. Split the
        # first-needed tensors into per-k-chunk DMAs so the first projection
        # matmuls unblock as soon as their own 64-128KB chunk lands instead
        # of waiting for the full 512KB tile.
        w_sb = singles.tile([128, 3, 4, 512], f16, tag="w")
        xt0_tile = p_xt.tile([128, 2, 4, 256], f16, tag="xt", name="xt0t")
        bqk_sb = singles.tile([128, 8], f32, tag="bqk")
        bm_sb = singles.tile([128, 128], f16, tag="bm")
        id_sb = singles.tile([128, 128], f16, tag="id")
        wo_sb = singles.tile([128, 4, D], f16, tag="wo")
        for k in range(4):
            nc.scalar.dma_start(out=w_sb[:, 0, k], in_=w_d[:, 0, k])  # Q
            nc.sync.dma_start(out=w_sb[:, 1, k], in_=w_d[:, 1, k])  # K
            for bb in range(2):
                nc.gpsimd.dma_start(out=xt0_tile[:, bb, k], in_=xt_d[bb, :, k])
            if k == 0:
                nc.gpsimd.dma_start(out=bqk_sb, in_=bqk_d)
                nc.gpsimd.dma_start(out=bm_sb, in_=bm_d)
                nc.gpsimd.dma_start(out=id_sb, in_=id_d)
        for k in range(4):
            nc.scalar.dma_start(out=w_sb[:, 2, k], in_=w_d[:, 2, k])  # V
        nc.gpsimd.dma_start(out=wo_sb, in_=wo_d)

        # persistent V tiles (pair double-buffer x batch): ones written once
        vas_db = []
        for i in range(2):
            row = []
            for j in range(2):
                va = singles.tile([128, 2, 8, 66], f16, tag=f"va{i}_{j}", name="va")
                nc.vector.memset(
                    va[:, :, :, 64:66].bitcast(mybir.dt.uint32), 0x3C003C00
                )
                row.append(va)
            vas_db.append(row)

        # Warm-up matmuls: these execute only after the engine-init preamble
        # (same point the input DMAs start flowing), so they purely trade
        # delay for DMA-wait coverage. A handful covers the ~1.5us until the
        # first per-k weight/x chunks land; more would push real work out.
        zwu = singles.tile([128, 512], f16, tag="zwu")
        nc.vector.memset(zwu, 0.0)
        for _ in range(4):
            zps = psC.tile([128, 512], f32, tag="c", name="zps")
            nc.tensor.matmul(zps, lhsT=zwu[:, 0:128], rhs=zwu, start=True, stop=True)

        def load_xt(p, eng=None):
            t = p_xt.tile([128, 2, 4, 256], f16, tag="xt", name="xtt")
            e = eng if eng is not None else nc.sync
            for bb in range(2):
                e.dma_start(out=t[:, bb], in_=xt_d[p * 2 + bb])
            return t

        xts = {0: xt0_tile}
        if npairs > 1:
            xts[1] = load_xt(1)

        # ---- work-item helpers (each = one N=512 projection group) ----
        def qk_group(f, qkt, xt):
            qp = psC.tile([128, 2, 256], f32, tag="c")
            for k in range(4):
                nc.tensor.matmul(
                    qp,
                    lhsT=w_sb[:, f // 4, k, (f % 4) * 128 : (f % 4 + 1) * 128],
                    rhs=xt[:, :, k, :],
                    start=(k == 0),
                    stop=(k == 3),
                )
            if f % 2 == 0:
                nc.scalar.add(qkt[:, f], qp, bqk_sb[:, f : f + 1])
            else:
                nc.vector.tensor_scalar_add(qkt[:, f], qp, bqk_sb[:, f : f + 1])

        def v_group(bb, st, va, xt):
            vp = psC.tile([128, 512], f32, tag="c")
            for k in range(4):
                nc.tensor.matmul(
                    vp,
                    lhsT=xt[:, bb, k, st * 128 : (st + 1) * 128],
                    rhs=w_sb[:, 2, k, :],
                    start=(k == 0),
                    stop=(k == 3),
                )
            nc.scalar.activation(
                va[:, st, :, 0:64],
                vp.rearrange("p (h c) -> p h c", c=64),
                AF.Copy,
            )

        def outproj_mms(yp, otsb, tt, fs, start, stop):
            for i, f in enumerate(fs):
                nc.tensor.matmul(
                    yp,
                    lhsT=otsb[:, f, tt * 128 : (tt + 1) * 128],
                    rhs=wo_sb[:, f, :],
                    start=(start and i == 0),
                    stop=(stop and i == len(fs) - 1),
                )

        def outproj_store(p, bb, tt, yp):
            # f16 output store; the output bias beff is added on the host.
            # Split the PSUM->SBUF downconvert across DVE and ACT halves so
            # the PSUM tile recycles in ~350ns instead of ~690ns.
            ysb = p_y.tile([128, 512], f16, tag="y")
            nc.vector.tensor_copy(out=ysb[:, 0:256], in_=yp[:, 0:256])
            nc.scalar.activation(ysb[:, 256:512], yp[:, 256:512], AF.Copy)
            qeng = nc.sync if (bb + tt) % 2 == 0 else nc.gpsimd
            qeng.dma_start(
                out=y_d[p * 2 + bb, tt * 128 : (tt + 1) * 128, :], in_=ysb
            )

        def outproj_group(p, bb, tt, otsb):
            yp = psC.tile([128, 512], f32, tag="c")
            outproj_mms(yp, otsb, tt, range(4), True, True)
            outproj_store(p, bb, tt, yp)

        def run_item(item):
            kind = item[0]
            if kind == "qk":
                _, f, qkt, xt = item
                qk_group(f, qkt, xt)
            elif kind == "v":
                _, bb, st, va, xt = item
                v_group(bb, st, va, xt)
            else:
                _, p, bb, tt, otsb = item
                outproj_group(p, bb, tt, otsb)

        # Micro-op form of the projection groups: a list of (is_mm, thunk)
        # entries (4 matmuls + 1 off-PE finisher) so single projection
        # matmuls can be laced between the small attention matmuls. Each
        # small matmul's ~107ns LDWEIGHTS then overlaps a 512-col stream
        # via the PE's reorder window instead of serializing.
        def proj_thunks(item):
            kind = item[0]
            box = {}
            ths = []
            if kind == "qk":
                _, f, qkt_, xt = item

                def mkq(k, f=f, xt=xt):
                    def th():
                        if k == 0:
                            box["ps"] = psC.tile([128, 2, 256], f32, tag="c")
                        nc.tensor.matmul(
                            box["ps"],
                            lhsT=w_sb[:, f // 4, k, (f % 4) * 128 : (f % 4 + 1) * 128],
                            rhs=xt[:, :, k, :],
                            start=(k == 0),
                            stop=(k == 3),
                        )
                    return th

                ths = [(True, mkq(k)) for k in range(4)]

                def finq(f=f, qkt_=qkt_):
                    if f % 2 == 0:
                        nc.scalar.add(qkt_[:, f], box["ps"], bqk_sb[:, f : f + 1])
                    else:
                        nc.vector.tensor_scalar_add(
                            qkt_[:, f], box["ps"], bqk_sb[:, f : f + 1]
                        )

                ths.append((False, finq))
            elif kind == "v":
                _, bb, st, va, xt = item

                def mkv(k, bb=bb, st=st, xt=xt):
                    def th():
                        if k == 0:
                            box["ps"] = psC.tile([128, 512], f32, tag="c")
                        nc.tensor.matmul(
                            box["ps"],
                            lhsT=xt[:, bb, k, st * 128 : (st + 1) * 128],
                            rhs=w_sb[:, 2, k, :],
                            start=(k == 0),
                            stop=(k == 3),
                        )
                    return th

                ths = [(True, mkv(k)) for k in range(4)]

                def finv(st=st, va=va):
                    vr = box["ps"].rearrange("p (h c) -> p h c", c=64)
                    nc.scalar.activation(va[:, st, 0:4, 0:64], vr[:, 0:4], AF.Copy)
                    nc.vector.tensor_copy(out=va[:, st, 4:8, 0:64], in_=vr[:, 4:8])

                ths.append((False, finv))
            else:
                _, pp, bb, tt, otsb = item

                def mko(f, otsb=otsb, tt=tt):
                    def th():
                        if f == 0:
                            box["ps"] = psC.tile([128, 512], f32, tag="c")
                        nc.tensor.matmul(
                            box["ps"],
                            lhsT=otsb[:, f, tt * 128 : (tt + 1) * 128],
                            rhs=wo_sb[:, f, :],
                            start=(f == 0),
                            stop=(f == 3),
                        )
                    return th

                ths = [(True, mko(f)) for f in range(4)]
                ths.append(
                    (False, lambda pp=pp, bb=bb, tt=tt: outproj_store(
                        pp, bb, tt, box["ps"]))
                )
            return ths

        class Feeder:
            def __init__(self):
                self.pending = []
                self.emitted = 0

            def push(self, item):
                self.pending += proj_thunks(item)

            def emit_until(self, target_mms):
                while self.emitted < target_mms and self.pending:
                    is_mm, th = self.pending.pop(0)
                    th()
                    if is_mm:
                        self.emitted += 1
                while self.pending and not self.pending[0][0]:
                    self.pending.pop(0)[1]()

            def flush(self):
                while self.pending:
                    self.pending.pop(0)[1]()

        # ---- attention stage helpers (sched form: list of (is_mm, thunk)) --
        def s_sched(s, qkt):
            fp, bb = s // 2, s % 2
            sps = [psB.tile([128, 384], f32, tag="s") for _ in range(2)]
            sch = []
            for st in range(2):
                for hh in range(2):
                    po = hh * 64
                    qt = qkt[po : po + 64, fp, bb, :]
                    kt = qkt[po : po + 64, 4 + fp, bb, :]
                    if st == 0:
                        sch.append((True, lambda sp=sps[hh], kt=kt, qt=qt:
                            nc.tensor.matmul(
                                sp[:, 0:256], lhsT=kt[:, 0:128], rhs=qt,
                                start=True, stop=True,
                            )))
                    else:
                        sch.append((True, lambda sp=sps[hh], kt=kt, qt=qt:
                            nc.tensor.matmul(
                                sp[:, 256:384], lhsT=kt[:, 128:256],
                                rhs=qt[:, 128:256], start=True, stop=True,
                            )))
            ets = [p_et.tile([128, 384], f16, tag="et") for _ in range(2)]

            def exp_fin():
                for hh in range(2):
                    nc.scalar.activation(ets[hh], sps[hh], AF.Exp)
                    dv = ets[hh].rearrange("p (a c) -> p a c", a=3)[:, 0::2, :]
                    nc.gpsimd.tensor_mul(
                        out=dv, in0=dv,
                        in1=bm_sb[:, None, :].broadcast_to([128, 2, 128]),
                    )

            sch.append((False, exp_fin))
            return ets, sch

        def o_sched(s, ets, va):
            fp = s // 2
            osb = p_o.tile([128, 2, 128], f16, tag="o")
            sch = []
            for hh in range(2):
                h = 2 * fp + hh
                po = hh * 64
                et = ets[hh]
                box = {}

                def m1(et=et, h=h, box=box):
                    box["op"] = psA.tile([128, 2, 66], f32, tag="a")
                    nc.tensor.matmul(
                        box["op"][:, 0, :], lhsT=et[:, 0:128], rhs=va[:, 0, h, :],
                        start=True, stop=True,
                    )

                def m2(et=et, h=h, box=box):
                    nc.tensor.matmul(
                        box["op"][:, 1, :], lhsT=et[:, 128:256], rhs=va[:, 0, h, :],
                        start=True, stop=False,
                    )

                def m3(et=et, h=h, box=box):
                    nc.tensor.matmul(
                        box["op"][:, 1, :], lhsT=et[:, 256:384], rhs=va[:, 1, h, :],
                        start=False, stop=True,
                    )

                def fin(po=po, box=box):
                    op = box["op"]
                    li = p_li.tile([128, 2], f32, tag="li")
                    nc.vector.reciprocal(li, op[:, :, 64])
                    nc.vector.tensor_mul(
                        out=osb[:, :, po : po + 64],
                        in0=op[:, :, 0:64],
                        in1=li[:, :, None].broadcast_to([128, 2, 64]),
                    )

                sch += [(True, m1), (True, m2), (True, m3), (False, fin)]
            return osb, sch

        def t_sched(s, osb, otsb, eng_sel):
            fp = s // 2
            otp = psB.tile([128, 2, 128], f16, tag="s")
            sch = [
                (True, lambda tt=tt: nc.tensor.transpose(
                    otp[:, tt, :], osb[:, tt, :], id_sb))
                for tt in range(2)
            ]
            sch.append(
                (False, lambda: nc.vector.tensor_copy(out=otsb[:, fp, :], in_=otp))
            )
            return sch

        def run_sched(sch):
            for _, th in sch:
                th()

        def o_mms(s, ets, va):
            osb, sch = o_sched(s, ets, va)
            run_sched(sch)
            return osb

        def t_mms(s, osb, otsb, eng_sel):
            run_sched(t_sched(s, osb, otsb, eng_sel))

        # ---- prologue: QK(0) + V(0) ----
        qkt0 = p_qkt.tile([128, 8, 2, 256], f16, tag="qkt", name="qkt0")
        qkts = {0: qkt0}
        for f in (0, 1, 2, 3, 4, 5, 6, 7):
            qk_group(f, qkts[0], xts[0])
        for bb in range(2):
            for st in range(2):
                v_group(bb, st, vas_db[0][bb], xts[0])

        otsbs_by_pair = {}
        tpend = []  # transposes pending (depth-2 pipeline, crosses pairs)

        # ---- main loop ----
        for p in range(npairs):
            last = p == npairs - 1
            qkt = qkts.pop(p)
            va_pair = vas_db[p % 2]
            otsbs = [
                p_ot.tile([128, 4, 256], f16, tag="ot", name=f"ot{p}_{i}")
                for i in range(2)
            ]
            otsbs_by_pair[p] = otsbs

            # work queue of projection groups to interleave into this pair
            Q = []
            if p + 1 < npairs:
                qkts[p + 1] = p_qkt.tile([128, 8, 2, 256], f16, tag="qkt", name=f"qkt{p+1}")
                for f in range(8):
                    Q.append(("qk", f, qkts[p + 1], xts[p + 1]))
                for bb in range(2):
                    for st in range(2):
                        Q.append(("v", bb, st, vas_db[(p + 1) % 2][bb], xts[p + 1]))
                if p + 2 < npairs:
                    xts[p + 2] = load_xt(p + 2)
            for pp in ([p - 2] if p - 2 >= 0 else []):
                for bb in range(2):
                    for tt in range(2):
                        Q.append(("out", pp, bb, tt, otsbs_by_pair[pp][bb]))
            if last and p - 1 >= 0:
                for bb in range(2):
                    for tt in range(2):
                        Q.append(("out", p - 1, bb, tt, otsbs_by_pair[p - 1][bb]))
            if p - 3 in otsbs_by_pair:
                del otsbs_by_pair[p - 3]

            # Last pair: run batch-0 slots first so the batch-0 out-projection
            # can start while batch-1 attention is still in flight; its two
            # groups are held until t(6) has landed (position >= 6).
            order = [0, 2, 4, 6, 1, 3, 5, 7] if last else list(range(8))
            Qlate = []
            if last:
                for tt in range(2):
                    Qlate.append(("out", p, 0, tt, otsbs[0]))

            qi = 0  # queue cursor
            prev = None  # slot state awaiting O
            for pos, s in enumerate(order):
                # transpose from two slots back (osb guaranteed normalized)
                if tpend:
                    t_mms(*tpend.pop(0))
                sps = s_mms(s, qkt)
                ets = exp_mask(sps)

                # first projection group
                qtarget = (len(Q) * (2 * pos + 1)) // 16
                while qi < qtarget:
                    run_item(Q[qi])
                    qi += 1
                # O for previous slot
                if prev is not None:
                    ps, pets = prev
                    osb = o_mms(ps, pets, va_pair[ps % 2])
                    tpend.append((ps, osb, otsbs[ps % 2], ps % 2 == 0))
                qtarget = (len(Q) * (2 * pos + 2)) // 16
                while qi < qtarget:
                    run_item(Q[qi])
                    qi += 1
                if pos >= 6 and Qlate:
                    run_item(Qlate.pop(0))
                prev = (s, ets)

            # flush the final slot
            while qi < len(Q):
                run_item(Q[qi])
                qi += 1
            while Qlate:
                run_item(Qlate.pop(0))
            ps, pets = prev
            osb = o_mms(ps, pets, va_pair[ps % 2])
            tpend.append((ps, osb, otsbs[ps % 2], ps % 2 == 0))

        # ---- epilogue: keep the PE queue dense to the end. tpend holds
        # t(5) and t(7) of the last pair; otsb[1] sections f0/f1 are already
        # final, f2 lands with t(5), f3 with t(7). Start the batch-1
        # out-projections on f0..f2 before the last transpose, finish with
        # the f3 chunk after it so only one 512-col matmul trails t(7).
        p = npairs - 1
        otsb1 = otsbs_by_pair[p][1]
        if len(tpend) == 2:
            t_mms(*tpend.pop(0))  # t(5): completes otsb[1] f2
            yps = []
            for tt in range(2):
                yp = psC.tile([128, 512], f32, tag="c", name=f"ypl{tt}")
                outproj_mms(yp, otsb1, tt, (0, 1, 2), True, False)
                yps.append(yp)
            t_mms(*tpend.pop(0))  # t(7): completes otsb[1] f3
            for tt in range(2):
                outproj_mms(yps[tt], otsb1, tt, (3,), False, True)
                outproj_store(p, 1, tt, yps[tt])
        else:
            # sim/debug fallback (npairs == 1 keeps the same shape: 2 pends)
            while tpend:
                t_mms(*tpend.pop(0))
            for tt in range(2):
                outproj_group(p, 1, tt, otsb1)

    nc.compile()
    return nc


def host_inputs(x, W_qkv, b_qkv, W_out, b_out):
    """Host-side preprocessing. Returns per-core-shared inputs plus the
    transposed x layout [B, 128, 4, 256] (d-major tiles)."""
    scale = 1.0 / np.sqrt(HD)
    W = np.array(W_qkv, dtype=np.float32).copy()
    W[:, :D] *= scale  # fold attention scale into Q projection
    bq = np.array(b_qkv, dtype=np.float64).copy()
    bq[:D] *= scale
    bqk = np.stack([bq[j * 128 : (j + 1) * 128] for j in range(8)], axis=1).astype(
        np.float32
    )
    beff_row = (
        np.array(b_qkv[2 * D :], np.float64) @ np.array(W_out, np.float64)
        + np.array(b_out, np.float64)
    ).astype(np.float32)
    i = np.arange(128)[:, None]
    j = np.arange(128)[None, :]
    binm = (j >= i).astype(np.float32)  # 1 on/above diagonal (t >= s)
    ident = np.eye(128, dtype=np.float32)
    # device weight layouts: partition-major so DMAs land contiguous lines
    # wqkv [128, 3, 4, 512]: [p, sec, k, n] = W[k*128+p, sec*512+n]
    wdev = (
        W.reshape(4, 128, 3, 512).transpose(1, 2, 0, 3).astype(np.float16)
    )
    wodev = (
        np.array(W_out, np.float32).reshape(4, 128, 512).transpose(1, 0, 2)
    ).astype(np.float16)
    shared = {
        "wqkv": np.ascontiguousarray(wdev),
        "wout": np.ascontiguousarray(wodev),
        "bqk": bqk,
        "binm": binm.astype(np.float16),
        "ident": ident.astype(np.float16),
    }
    return shared, beff_row


def xt_layout(x):
    """[B, T, D] -> [B, 128, 4, 256]: xt[b, p, k, t] = x[b, t, 128k+p]."""
    xb = np.asarray(x, dtype=np.float32)
    return np.ascontiguousarray(
        xb.transpose(0, 2, 1).reshape(-1, 4, 128, T).transpose(0, 2, 1, 3)
    ).astype(np.float16)


def kernel(x, W_qkv, b_qkv, W_out, b_out):
    from concourse.bass_utils import run_bass_kernel_spmd

    shared, beff_row = host_inputs(x, W_qkv, b_qkv, W_out, b_out)
    xt = xt_layout(x)
    nc = build_nc(BL, NCORES)
    in_maps = [
        {"xt": xt[c * BL : (c + 1) * BL], **shared} for c in range(NCORES)
    ]
    res = run_bass_kernel_spmd(nc, in_maps, core_ids=list(range(NCORES)))
    y = np.concatenate([res.results[c]["y"] for c in range(NCORES)], axis=0)
    return y.astype(np.float32) + beff_row

